# revision 1
# baseline (speedup 1.0000x reference)
"""Trainium2 Bass kernel for nn_GCN_18820546691816.

The GCN collapses to a per-row MLP chain applied to x1 [B, 112]:
    h1 = relu(x1 @ M1 + b1v)    M1 = kron(A^T, W1)  [112, 56]
    h2 = relu(h1 @ M2 + b2v)    M2 = kron(A^T, W2)  [56, 56]
    h3 = relu(h2 @ Wl1 + bl1)   [56, 24]
    y  = h3 @ Wl2 + bl2         [24, 1]

Device mapping (per core, batch features-on-partitions):
  - All four layer weights occupy disjoint 32x32 subarray regions of the
    128x128 PE array via tile_position, so the four matmuls of a pipelined
    round can run concurrently:
        L1 at rows 0-111,  cols 0-55   (out -> PSUM parts 0-55)
        L2 at rows 0-55,   cols 64-119 (out -> PSUM parts 64-119)
        L3 at rows 64-119, cols 96-119 (out -> PSUM parts 96-119)
        L4 at rows 96-119, cols 64     (out -> PSUM part 64)
  - One ScalarE activation per round does relu+bias for L1 and L2 outputs
    together (disjoint partitions of one PSUM tile).
  - One VectorE tensor_scalar per round does relu+bias for L3 and the final
    +bl2 for L4 (the L4 row uses a -3e38 max-floor so it passes through).
  - x1 is transposed host-side during sharding so tiles load contiguously.
  - Matmuls run in fp16 (1 cycle/col; fp32 runs at 1/4 rate and float32r
    does not support PE subarray tiling). fp16's 10 mantissa bits keep the
    end-to-end error around 1e-3 relative.

Data-parallel over 8 cores: x1T sharded along batch, weights replicated.
"""

from contextlib import ExitStack

import numpy as np

import concourse.bass as bass
import concourse.tile as tile
from concourse import mybir
from concourse.tile_rust import add_dep_helper
from concourse.bass import ds
from concourse.bass_utils import run_bass_kernel_spmd

N_CORES = 8
B = 262144
F_IN = 112
BPC = B // N_CORES        # 32768 samples per core
NB = 1024                 # samples per super-round (2 PSUM banks)
HALF = 512                # matmul free-dim (one PSUM bank, fp32)
T = BPC // NB             # 32 super-rounds of real work
CH = 8                    # s_big output ring depth in rounds

F32 = mybir.dt.float32
F16 = mybir.dt.float16

# fp16 weight blob column layout:
#   [0:56)    M1   (rows 0-111)
#   [56:112)  M2   (rows 0-55)
#   [112:136) Wl1  (rows 64-119)
#   [136:137) Wl2  (rows 96-119)
WGT_COLS = 137
# float32 scalar blob columns: 0 = ACT bias, 1 = DVE bias, 2 = DVE floor
SCL_COLS = 3


def _norm_adj_np(edge_index):
    ei = np.asarray(edge_index)
    src = np.concatenate([ei[0], np.arange(7, dtype=ei.dtype)])
    dst = np.concatenate([ei[1], np.arange(7, dtype=ei.dtype)])
    deg = np.zeros(7, np.float32)
    np.add.at(deg, dst, np.float32(1.0))
    dinv = np.where(deg > 0, deg ** np.float32(-0.5), np.float32(0.0)).astype(
        np.float32
    )
    w = (dinv[src] * dinv[dst]).astype(np.float32)
    A = np.zeros((7, 7), np.float32)
    np.add.at(A, (dst, src), w)
    return A


def _pack_weights(A, W1, W2, Wl1, Wl2):
    M1 = np.kron(A.T, np.asarray(W1)).astype(np.float32)  # [112, 56]
    M2 = np.kron(A.T, np.asarray(W2)).astype(np.float32)  # [56, 56]
    blob = np.zeros((128, WGT_COLS), np.float32)
    blob[0:112, 0:56] = M1
    blob[0:56, 56:112] = M2
    blob[64:120, 112:136] = np.asarray(Wl1, np.float32)
    blob[96:120, 136:137] = np.asarray(Wl2, np.float32)
    return blob.astype(np.float16)


def _pack_scalars(b1, b2, bl1, bl2):
    blob = np.zeros((128, SCL_COLS), np.float32)
    # ACT bias vector: parts 0-55 get b1 (tiled over nodes), 64-119 get b2
    blob[0:56, 0] = np.tile(np.asarray(b1, np.float32), 7)
    blob[64:120, 0] = np.tile(np.asarray(b2, np.float32), 7)
    # DVE scalars for PSUM-B post-op on parts 64-119:
    #   part 64  (L4 out): + bl2, floor -3e38 (no-op relu)
    #   parts 96-119 (L3 out): + bl1, floor 0 (relu)
    blob[64, 1] = np.float32(np.asarray(bl2).reshape(-1)[0])
    blob[96:120, 1] = np.asarray(bl1, np.float32)
    blob[64, 2] = np.float32(-3.0e38)
    return blob


def _split_multiwaits(nc):
    """Walrus accepts only one sync wait per lowered instruction; hoist all
    but the last wait of any multi-wait instruction onto single-wait NOPs
    placed immediately before it on the same engine (engines execute their
    stream in order, so the NOP chain is equivalent)."""
    for f in nc.m.functions:
        for bb in f.blocks:
            out = []
            changed = False
            for inst in bb.instructions:
                si = inst.sync_info
                if si is not None and si.on_wait and len(si.on_wait) > 1:
                    waits = list(si.on_wait)
                    for w in waits[:-1]:
                        nop = mybir.InstNoOp(
                            name=nc.get_next_instruction_name(),
                            engine=inst.engine,
                            sync_info=mybir.SyncInfo(on_wait=[w], on_update=[]),
                            text_hint="split_wait",
                            bass_nofuse=True,
                        )
                        out.append(nop)
                    inst.sync_info = mybir.SyncInfo(
                        on_wait=[waits[-1]], on_update=list(si.on_update or [])
                    )
                    changed = True
                out.append(inst)
            if changed:
                bb.instructions = out


def _build_nc():
    nc = bass.Bass("TRN2", target_bir_lowering=False, debug=False)
    xT = nc.dram_tensor("xT", [F_IN, BPC], F16, kind="ExternalInput").ap()
    wgt = nc.dram_tensor("wgt", [128, WGT_COLS], F16, kind="ExternalInput").ap()
    scl = nc.dram_tensor("scl", [128, SCL_COLS], F32, kind="ExternalInput").ap()
    # One output tensor per 8-round window cycle: separate tensors so the
    # final DMAs carry no WAW chain (walrus allows one sync wait per
    # instruction, and a DMA-completion wait must be the only one). Window w
    # of chunk k holds block 8k + ((w - 5) % 8); the host undoes the
    # permutation.
    ys = [
        nc.dram_tensor(f"y{k}", [1, CH * NB], F16, kind="ExternalOutput").ap()
        for k in range(T // CH)
    ]

    with tile.TileContext(nc) as tc, ExitStack() as ctx:
        wpool = ctx.enter_context(tc.tile_pool(name="wpool", bufs=1))
        # One slot per xt tile: no slot reuse means the xt DMAs carry no
        # WAR/WAW semaphore waits at all. 32 x 4KB/partition = 16 MB of SBUF.
        xpool = ctx.enter_context(tc.tile_pool(name="xpool", bufs=T))
        hpool = ctx.enter_context(tc.tile_pool(name="hpool", bufs=4))
        # Persistent ping-pong PSUM tiles (not pool-rotated): slot releases
        # are what force un-elidable PE self-waits on the first writer of a
        # reused slot, and plain same-tile WAW on one engine needs no sem.
        ps_pool = ctx.enter_context(tc.tile_pool(name="ps", bufs=1, space="PSUM"))

        wb = wpool.tile([128, WGT_COLS], F16)
        nc.sync.dma_start(wb[:, :], wgt)
        sb = wpool.tile([128, SCL_COLS], F32)
        nc.sync.dma_start(sb[:, :], scl)
        w1 = wb[0:112, 0:56]
        w2 = wb[0:56, 56:112]
        w3 = wb[64:120, 112:136]
        w4 = wb[96:120, 136:137]
        actbias = sb[0:120, 0]
        sbias = sb[64:120, 1]
        sfloor = sb[64:120, 2]

        relu = mybir.ActivationFunctionType.Relu
        add_op = mybir.AluOpType.add
        max_op = mybir.AluOpType.max

        xt = {}   # t -> xT tile [112, NB] f32r
        h = {}    # t -> h tile [128, NB] f32r: [0:56]=h1(t), [64:120]=h2(t-2)
        pA_pp = [ps_pool.tile([128, NB], F32, name=f"pApp{i}", tag=f"pA{i}")
                 for i in range(2)]
        pB_pp = [ps_pool.tile([128, NB], F32, name=f"pBpp{i}", tag=f"pB{i}")
                 for i in range(2)]

        # s ring: round t uses column window t % CH. Partition 64 of window
        # w(t) = y(t-5); partitions 96-119 = h3(t-3). One gpsimd DMA per CH
        # rounds ships the whole partition-64 row.
        s_big = wpool.tile([128, CH * NB], F16)

        def s_win(t):
            return s_big[:, ds((t % CH) * NB, NB)]

        # Engine "clock pumps": walrus accepts a single sync wait per
        # instruction, and engines do not observe their own semaphore ticks.
        # A 1x1 op at the end of each round waits on its own engine's
        # previous pump tick, which (a) is itself a legal single wait and
        # (b) advances the engine's observed self-tick past every
        # same-engine hazard from earlier rounds, so the real instructions
        # carry only their single cross-engine data wait.
        act_scr = wpool.tile([1, 1], F32)
        dve_scr = wpool.tile([1, 1], F32)
        # preamble: absorb the wgt/scl DMA lane ticks per engine
        nc.tensor.matmul(pA_pp[0][96:97, 0:1], wb[0:1, 0:1], wb[0:1, 0:1],
                         start=True, stop=True, tile_position=(0, 96))
        nc.scalar.copy(act_scr[0:1, 0:1], sb[0:1, 0:1])
        nc.vector.tensor_copy(dve_scr[0:1, 0:1], sb[0:1, 1:2])

        # Pipeline lags: L1 block t at round t; L2 at t+2; L3 at t+3; L4 at
        # t+5. Every PE instruction reads data produced >= 1 round earlier,
        # so the PE never stalls on the current round's ACT/DVE. Emission
        # order per round keeps each matmul at one new semaphore wait.
        for t in range(T + 5):
            if t < T:
                xt[t] = xpool.tile([F_IN, NB], F16, name=f"xt{t}", tag="xt")
                nc.sync.dma_start(xt[t][:, :], xT[:, ds(t * NB, NB)])
            pA = pA_pp[t % 2]
            pB = pB_pp[t % 2]

            last_pe = last_act = last_dve = None

            if t >= 13 and (t - 13) % CH == 0:
                # absorb the latest out-DMA lane tick into the DVE clock
                # (window 0 already shipped; safe to scribble)
                last_dve = nc.vector.memset(s_big[64:65, 0:1], 0.0)

            for j in range(NB // HALF):
                c = ds(j * HALF, HALF)
                if 2 <= t <= T + 1:  # L2(t-2)
                    last_pe = nc.tensor.matmul(
                        pA[64:120, c], w2, h[t - 2][0:56, c],
                        start=True, stop=True, tile_position=(0, 64),
                    )
            h.pop(t - 2, None)
            for j in range(NB // HALF):
                c = ds(j * HALF, HALF)
                if t < T:  # L1(t)
                    last_pe = nc.tensor.matmul(
                        pA[0:56, c], w1, xt[t][:, c],
                        start=True, stop=True, tile_position=(0, 0),
                    )
            xt.pop(t, None)
            for j in range(NB // HALF):
                c = ds(j * HALF, HALF)
                if 5 <= t:  # L4(t-5)
                    last_pe = nc.tensor.matmul(
                        pB[64:65, c], w4, s_win(t - 2)[96:120, c],
                        start=True, stop=True, tile_position=(96, 64),
                    )
            for j in range(NB // HALF):
                c = ds(j * HALF, HALF)
                if 3 <= t <= T + 2:  # L3(t-3)
                    last_pe = nc.tensor.matmul(
                        pB[96:120, c], w3, h[t - 1][64:120, c],
                        start=True, stop=True, tile_position=(64, 96),
                    )

            if t <= T + 1:
                h[t] = hpool.tile([128, NB], F16, name=f"h{t}", tag="h")
                last_act = nc.scalar.activation(
                    h[t][0:120, :], pA[0:120, :], relu, bias=actbias[:, None]
                )
            if t >= 3:
                last_dve = nc.vector.tensor_scalar(
                    s_win(t)[64:120, :], pB[64:120, :],
                    sbias[:, None], sfloor[:, None], add_op, max_op,
                )
            if t >= 12 and (t - 12) % CH == 0:
                nc.gpsimd.dma_start(ys[(t - 12) // CH][:, :], s_big[64:65, :])

            # End-of-round engine clock pumps, order-pinned (sync=False)
            # behind the round's last real op so the scheduler cannot hoist
            # them. Each pump's only semaphore wait is its own engine's
            # previous pump tick, which advances the engine's observed
            # self-clock past every same-engine hazard from earlier rounds.
            if last_act is not None:
                p = nc.scalar.copy(act_scr[0:1, 0:1], act_scr[0:1, 0:1])
                add_dep_helper(p.ins, last_act.ins, sync=False, reason="pin act pump")
            if last_dve is not None:
                p = nc.vector.tensor_copy(dve_scr[0:1, 0:1], dve_scr[0:1, 0:1])
                add_dep_helper(p.ins, last_dve.ins, sync=False, reason="pin dve pump")



    _split_multiwaits(nc)
    return nc


_NC_CACHE = None


def _get_nc():
    global _NC_CACHE
    if _NC_CACHE is None:
        _NC_CACHE = _build_nc()
    return _NC_CACHE


def _make_in_maps(x1, edge_index, W1, b1, W2, b2, Wl1, bl1, Wl2, bl2):
    x1 = np.asarray(x1, np.float32)
    A = _norm_adj_np(edge_index)
    wgt = _pack_weights(A, W1, W2, Wl1, Wl2)
    scl = _pack_scalars(b1, b2, bl1, bl2)
    x1T = np.ascontiguousarray(x1.T.astype(np.float16))  # [112, B] fp16
    return [
        {
            "xT": np.ascontiguousarray(x1T[:, c * BPC : (c + 1) * BPC]),
            "wgt": wgt,
            "scl": scl,
        }
        for c in range(N_CORES)
    ]


def kernel(x1, edge_index, W1, b1, W2, b2, Wl1, bl1, Wl2, bl2, **_unused):
    in_maps = _make_in_maps(x1, edge_index, W1, b1, W2, b2, Wl1, bl1, Wl2, bl2)
    nc = _get_nc()
    res = run_bass_kernel_spmd(nc, in_maps, list(range(N_CORES)))
    return _gather_y(res.results)


def _gather_y(results):
    # window w of chunk k holds block 8k + ((w - 5) % 8): block b sits at
    # window (b + 5) % 8
    worder = [(b + 5) % CH for b in range(CH)]
    parts = []
    for c in range(N_CORES):
        for k in range(T // CH):
            yk = results[c][f"y{k}"].reshape(CH, NB)
            parts.append(yk[worder].reshape(-1))
    return np.concatenate(parts).reshape(B, 1).astype(np.float32)



# revision 11
# speedup vs baseline: 2.2959x; 2.2959x over previous
"""Trainium2 Bass kernel for nn_GCN_18820546691816.

The GCN collapses to a per-row MLP chain applied to x1 [B, 112]:
    h1 = relu(x1 @ M1 + b1v)    M1 = kron(A^T, W1)  [112, 56]
    h2 = relu(h1 @ M2 + b2v)    M2 = kron(A^T, W2)  [56, 56]
    h3 = relu(h2 @ Wl1 + bl1)   [56, 24]
    y  = h3 @ Wl2 + bl2         [24, 1]

PE issue cost on TRN2 is (moving free size) cycles per matmul regardless of
partition counts, so the design packs 2 samples per issued column wherever
the contraction fits in 128 partitions:
  - L1 (contraction 112/sample, unpackable): two matmuls per tile with the
    same M1 weights loaded at PE subarray columns 0 and 64, writing psum
    parts 0:64 (pair-even samples) and 64:128 (pair-odd). One ACT pass over
    the stacked [128, 512] psum emits h1 "pair-packed" [128 parts, 512].
  - L2: block-diag(M2, M2) [128, 112] consumes pair-packed h1 -> 0.5
    col/sample. Pool pass -> h2p [112, 512] per tile.
  - L3: block-diag(Wl1, Wl1) [112, 48->64] at subarray cols 0 and 64 over
    two adjacent h2 tiles -> quad-packed psum [128, 512]; DVE pass -> h3q.
  - L4: 4-sample-packed Wl2 [128, 4] -> y [4, 512] = 0.25 col/sample.
Total ~2.25 PE columns/sample vs 4 for the unpacked baseline, and the
fused tile schedule keeps the PE dense so it ramps to the 2.4 GHz p-state.
Host-side, x1 is pre-permuted so every matmul and engine pass reads/writes
contiguous APs; the final y [4, NQ*512] un-permutes with one transpose.

Data-parallel over 8 cores: batch sharded, weights replicated.
"""

from contextlib import ExitStack

import numpy as np

import concourse.bass as bass
import concourse.tile as tile
from concourse import mybir
from concourse.bass import ds
from concourse.bass_utils import run_bass_kernel_spmd

N_CORES = 8
B = 262144
F_IN = 112
BPC = B // N_CORES        # 32768 samples per core
NTILE = BPC // 1024       # 32 L1/L2 tiles (1024 samples each)
NQ = NTILE // 2           # 16 L3/L4 blocks (2048 samples each)
CHUNK_Q = 2               # L3-blocks per scheduling chunk (LDWEIGHTS batching)

F32 = mybir.dt.float32
F16 = mybir.dt.float16

# fp16 weight blob column layout [128 x 244]:
#   [0:64)     M1p    (rows 0:112; cols 56:64 zero)
#   [64:176)   W2bd   (block-diag M2 at [0:56,64:120) / [64:120,120:176))
#   [176:240)  Wl1bd  (rows 0:112; block-diag Wl1; cols 224:240 zero)
#   [240:244)  Wl2q   (4-sample-packed Wl2)
WGT_COLS = 244
# fp32 scalar blob columns: 0=h1 bias, 1=h2 bias, 2=zeros(floor), 3=h3 bias
# (h3 bias row 48 = 1.0 -> constant ones-row in h3q carrying bl2 through L4)
SCL_COLS = 4


def _norm_adj_np(edge_index):
    ei = np.asarray(edge_index)
    src = np.concatenate([ei[0], np.arange(7, dtype=ei.dtype)])
    dst = np.concatenate([ei[1], np.arange(7, dtype=ei.dtype)])
    deg = np.zeros(7, np.float32)
    np.add.at(deg, dst, np.float32(1.0))
    dinv = np.where(deg > 0, deg ** np.float32(-0.5), np.float32(0.0)).astype(
        np.float32
    )
    w = (dinv[src] * dinv[dst]).astype(np.float32)
    A = np.zeros((7, 7), np.float32)
    np.add.at(A, (dst, src), w)
    return A


def _pack_weights(A, W1, W2, Wl1, Wl2, bl2):
    M1 = np.kron(A.T, np.asarray(W1)).astype(np.float32)  # [112, 56]
    M2 = np.kron(A.T, np.asarray(W2)).astype(np.float32)  # [56, 56]
    Wl1 = np.asarray(Wl1, np.float32)
    Wl2 = np.asarray(Wl2, np.float32)
    blob = np.zeros((128, WGT_COLS), np.float32)
    blob[0:112, 0:56] = M1
    blob[0:56, 64:120] = M2
    blob[64:120, 120:176] = M2
    blob[0:56, 176:200] = Wl1
    blob[56:112, 200:224] = Wl1
    blob[0:24, 240] = Wl2[:, 0]
    blob[24:48, 241] = Wl2[:, 0]
    blob[64:88, 242] = Wl2[:, 0]
    blob[88:112, 243] = Wl2[:, 0]
    # h3q row 48 is forced to 1.0 by the dve3 bias; bl2 rides in that row so
    # the L4 matmul adds it and the y pass is a pure copy.
    blob[48, 240:244] = np.float32(np.asarray(bl2).reshape(-1)[0])
    return blob.astype(np.float16)


def _pack_scalars(b1, b2, bl1):
    b1 = np.asarray(b1, np.float32)
    b2 = np.asarray(b2, np.float32)
    bl1 = np.asarray(bl1, np.float32)
    blob = np.zeros((128, SCL_COLS), np.float32)
    blob[0:56, 0] = np.tile(b1, 7)
    blob[64:120, 0] = np.tile(b1, 7)
    blob[0:56, 1] = np.tile(b2, 7)
    blob[56:112, 1] = np.tile(b2, 7)
    blob[0:24, 3] = bl1
    blob[24:48, 3] = bl1
    blob[64:88, 3] = bl1
    blob[88:112, 3] = bl1
    blob[48, 3] = np.float32(1.0)  # h3q ones-row (carries bl2 via Wl2q)
    return blob


def _split_multiwaits(nc):
    """Walrus accepts only one sync wait per lowered instruction; hoist all
    but the last wait of any multi-wait instruction onto single-wait NOPs
    placed immediately before it on the same engine."""
    for f in nc.m.functions:
        for bb in f.blocks:
            out = []
            changed = False
            for inst in bb.instructions:
                si = inst.sync_info
                if si is not None and si.on_wait and len(si.on_wait) > 1:
                    waits = list(si.on_wait)
                    for w in waits[:-1]:
                        nop = mybir.InstNoOp(
                            name=nc.get_next_instruction_name(),
                            engine=inst.engine,
                            sync_info=mybir.SyncInfo(on_wait=[w], on_update=[]),
                            text_hint="split_wait",
                            bass_nofuse=True,
                        )
                        out.append(nop)
                    inst.sync_info = mybir.SyncInfo(
                        on_wait=[waits[-1]], on_update=list(si.on_update or [])
                    )
                    changed = True
                out.append(inst)
            if changed:
                bb.instructions = out


def _build_nc():
    nc = bass.Bass("TRN2", target_bir_lowering=False, debug=False)
    xin = nc.dram_tensor("xin", [F_IN, BPC], F16, kind="ExternalInput").ap()
    wgt = nc.dram_tensor("wgt", [128, WGT_COLS], F16, kind="ExternalInput").ap()
    scl = nc.dram_tensor("scl", [128, SCL_COLS], F32, kind="ExternalInput").ap()
    yout = nc.dram_tensor("y", [4, NQ * 512], F16, kind="ExternalOutput").ap()

    relu = mybir.ActivationFunctionType.Relu
    add_op = mybir.AluOpType.add
    max_op = mybir.AluOpType.max

    with tile.TileContext(nc) as tc, ExitStack() as ctx:
        wpool = ctx.enter_context(tc.tile_pool(name="wpool", bufs=1))
        xpool = ctx.enter_context(tc.tile_pool(name="xpool", bufs=NTILE))
        ps_pool = ctx.enter_context(tc.tile_pool(name="ps", bufs=1, space="PSUM"))

        wb = wpool.tile([128, WGT_COLS], F16)
        nc.sync.dma_start(wb[:, :], wgt)
        sb = wpool.tile([128, SCL_COLS], F32)
        nc.sync.dma_start(sb[:, :], scl)

        w_m1 = wb[0:112, 0:64]
        w_2 = wb[0:128, 64:176]
        w_3 = wb[0:112, 176:240]
        w_4 = wb[0:128, 240:244]
        b1v = sb[0:128, 0]
        b2v = sb[0:112, 1]
        zf2 = sb[0:112, 2]
        b3v = sb[0:128, 3]
        zf3 = sb[0:128, 2]

        # persistent SBUF intermediates (written in disjoint 512-col slices)
        h1p = wpool.tile([128, NTILE * 512], F16)
        h2p = wpool.tile([112, NTILE * 512], F16)
        h3q = wpool.tile([128, NQ * 512], F16)
        ysb = wpool.tile([4, NQ * 512], F16)

        # 8 persistent PSUM tiles = 8 banks, ping-pong per stage
        p1 = [ps_pool.tile([128, 512], F32, name=f"p1_{i}", tag=f"p1{i}")
              for i in range(2)]
        p2 = [ps_pool.tile([112, 512], F32, name=f"p2_{i}", tag=f"p2{i}")
              for i in range(2)]
        p3 = [ps_pool.tile([128, 512], F32, name=f"p3_{i}", tag=f"p3{i}")
              for i in range(2)]
        p4 = [ps_pool.tile([4, 512], F32, name=f"p4_{i}", tag=f"p4{i}")
              for i in range(2)]

        xt = {}
        NC_CHUNKS = NQ // CHUNK_Q

        def l1(t):
            xt[t] = xpool.tile([F_IN, 1024], F16, name=f"xt{t}", tag="xt")
            nc.sync.dma_start(xt[t][:, :], xin[:, ds(t * 1024, 1024)])
            pt = p1[t % 2]
            nc.tensor.matmul(pt[0:64, :], w_m1, xt[t][:, 0:512],
                             start=True, stop=True, tile_position=(0, 0))
            nc.tensor.matmul(pt[64:128, :], w_m1, xt[t][:, 512:1024],
                             start=True, stop=True, tile_position=(0, 64))

        def act1(t):
            nc.scalar.activation(
                h1p[:, ds(t * 512, 512)], p1[t % 2][:, :], relu,
                bias=b1v[:, None],
            )
            xt.pop(t, None)

        def l2(t):
            nc.tensor.matmul(p2[t % 2][:, :], w_2, h1p[:, ds(t * 512, 512)],
                             start=True, stop=True, tile_position=(0, 0))

        def dve2(t):
            nc.vector.tensor_scalar(
                h2p[:, ds(t * 512, 512)], p2[t % 2][:, :],
                b2v[:, None], zf2[:, None], add_op, max_op,
            )

        def l3(q):
            pq = p3[q % 2]
            nc.tensor.matmul(pq[0:64, :], w_3, h2p[:, ds(2 * q * 512, 512)],
                             start=True, stop=True, tile_position=(0, 0))
            nc.tensor.matmul(pq[64:128, :], w_3,
                             h2p[:, ds((2 * q + 1) * 512, 512)],
                             start=True, stop=True, tile_position=(0, 64))

        def dve3(q):
            nc.vector.tensor_scalar(
                h3q[:, ds(q * 512, 512)], p3[q % 2][:, :],
                b3v[:, None], zf3[:, None], add_op, max_op,
            )

        def l4(q):
            nc.tensor.matmul(p4[q % 2][:, :], w_4, h3q[:, ds(q * 512, 512)],
                             start=True, stop=True, tile_position=(0, 0))

        def act4(q):
            nc.scalar.activation(
                ysb[:, ds(q * 512, 512)], p4[q % 2][:, :],
                mybir.ActivationFunctionType.Copy,
            )
            if q % 4 == 3:
                nc.gpsimd.dma_start(
                    yout[:, ds((q - 3) * 512, 2048)],
                    ysb[:, ds((q - 3) * 512, 2048)],
                )

        # Fused schedule: per chunk C emit L1(C), L2(C-1), L3(C-2), L4(C-3).
        # Matmuls with the same weights+tile_position stay adjacent in the PE
        # stream (weight reloads at most 6x per chunk); each engine pass is
        # emitted right after its producing matmuls so the 2-deep PSUM
        # ping-pong WAR hazard is visible to the tile scheduler.
        for C in range(NC_CHUNKS + 3):
            qs = range(C * CHUNK_Q, (C + 1) * CHUNK_Q)
            if C < NC_CHUNKS:
                for q in qs:
                    for t in (2 * q, 2 * q + 1):
                        l1(t)
                        act1(t)
            if 1 <= C <= NC_CHUNKS:
                for q in qs:
                    for t in (2 * (q - CHUNK_Q), 2 * (q - CHUNK_Q) + 1):
                        l2(t)
                        dve2(t)
            if 2 <= C <= NC_CHUNKS + 1:
                for q in qs:
                    l3(q - 2 * CHUNK_Q)
                    dve3(q - 2 * CHUNK_Q)
            if 3 <= C <= NC_CHUNKS + 2:
                for q in qs:
                    l4(q - 3 * CHUNK_Q)
                    act4(q - 3 * CHUNK_Q)

    _split_multiwaits(nc)
    return nc


_NC_CACHE = None


def _get_nc():
    global _NC_CACHE
    if _NC_CACHE is None:
        _NC_CACHE = _build_nc()
    return _NC_CACHE


def _pack_x(x1):
    """Per-core [BPC, 112] -> [112, BPC] fp16 with the tile permutation:
    sample 4q+m of L3-block Q lands in L1-tile t=2Q+(m//2) at column
    (m%2)*512 + (q - 512Q)."""
    x1 = np.asarray(x1, np.float32)
    out = []
    for c in range(N_CORES):
        xc = x1[c * BPC:(c + 1) * BPC]
        v = xc.reshape(NQ, 512, 2, 2, F_IN)
        xin = v.transpose(0, 2, 3, 1, 4).reshape(BPC, F_IN)
        out.append(np.ascontiguousarray(xin.T.astype(np.float16)))
    return out


def _make_in_maps(x1, edge_index, W1, b1, W2, b2, Wl1, bl1, Wl2, bl2):
    A = _norm_adj_np(edge_index)
    wgt = _pack_weights(A, W1, W2, Wl1, Wl2, bl2)
    scl = _pack_scalars(b1, b2, bl1)
    xs = _pack_x(x1)
    return [{"xin": xs[c], "wgt": wgt, "scl": scl} for c in range(N_CORES)]


def kernel(x1, edge_index, W1, b1, W2, b2, Wl1, bl1, Wl2, bl2, **_unused):
    in_maps = _make_in_maps(x1, edge_index, W1, b1, W2, b2, Wl1, bl1, Wl2, bl2)
    nc = _get_nc()
    res = run_bass_kernel_spmd(nc, in_maps, list(range(N_CORES)))
    return _gather_y(res.results)


def _gather_y(results):
    parts = []
    for c in range(N_CORES):
        yc = results[c]["y"]  # [4, NQ*512]; col q part m = y(4q+m)
        parts.append(yc.T.reshape(-1))
    return np.concatenate(parts).reshape(B, 1).astype(np.float32)


# revision 15
# speedup vs baseline: 2.4325x; 1.0595x over previous
"""Trainium2 Bass kernel for nn_GCN_18820546691816.

The GCN collapses to a per-row MLP chain applied to x1 [B, 112]:
    h1 = relu(x1 @ M1 + b1v)    M1 = kron(A^T, W1)  [112, 56]
    h2 = relu(h1 @ M2 + b2v)    M2 = kron(A^T, W2)  [56, 56]
    h3 = relu(h2 @ Wl1 + bl1)   [56, 24]
    y  = h3 @ Wl2 + bl2         [24, 1]

PE issue cost on TRN2 is (moving free size) cycles per matmul regardless of
partition counts, so the design packs 2 samples per issued column wherever
the contraction fits in 128 partitions:
  - L1 (contraction 112/sample, unpackable): two matmuls per tile with the
    same M1 weights loaded at PE subarray columns 0 and 64, writing psum
    parts 0:64 (pair-even samples) and 64:128 (pair-odd). One ACT pass over
    the stacked [128, 512] psum emits h1 "pair-packed" [128 parts, 512].
  - L2: block-diag(M2, M2) [128, 112] consumes pair-packed h1 -> 0.5
    col/sample. Pool pass -> h2p [112, 512] per tile.
  - L3: block-diag(Wl1, Wl1) [112, 48->64] at subarray cols 0 and 64 over
    two adjacent h2 tiles -> quad-packed psum [128, 512]; DVE pass -> h3q.
  - L4: 4-sample-packed Wl2 [128, 4] -> y [4, 512] = 0.25 col/sample.
Total ~2.25 PE columns/sample vs 4 for the unpacked baseline, and the
fused tile schedule keeps the PE dense so it ramps to the 2.4 GHz p-state.
Host-side, x1 is pre-permuted so every matmul and engine pass reads/writes
contiguous APs; the final y [4, NQ*512] un-permutes with one transpose.

Data-parallel over 8 cores: batch sharded, weights replicated.
"""

from contextlib import ExitStack

import numpy as np

import concourse.bass as bass
import concourse.tile as tile
from concourse import mybir
from concourse.bass import ds
from concourse.bass_utils import run_bass_kernel_spmd

N_CORES = 8
B = 262144
F_IN = 112
BPC = B // N_CORES        # 32768 samples per core
NTILE = BPC // 1024       # 32 L1/L2 tiles (1024 samples each)
NQ = NTILE // 2           # 16 L3/L4 blocks (2048 samples each)
CHUNK_Q = 2               # L3-blocks per scheduling chunk (LDWEIGHTS batching)

F32 = mybir.dt.float32
F16 = mybir.dt.float16

# fp16 weight blob column layout [128 x 244]:
#   [0:64)     M1p    (rows 0:112; cols 56:64 zero)
#   [64:176)   W2bd   (block-diag M2 at [0:56,64:120) / [64:120,120:176))
#   [176:240)  Wl1bd  (rows 0:112; block-diag Wl1; cols 224:240 zero)
#   [240:244)  Wl2q   (4-sample-packed Wl2)
WGT_COLS = 244
# fp32 scalar blob columns: 0=h1 bias, 1=h2 bias, 2=zeros(floor), 3=h3 bias
# (h3 bias row 48 = 1.0 -> constant ones-row in h3q carrying bl2 through L4)
SCL_COLS = 4


def _norm_adj_np(edge_index):
    ei = np.asarray(edge_index)
    src = np.concatenate([ei[0], np.arange(7, dtype=ei.dtype)])
    dst = np.concatenate([ei[1], np.arange(7, dtype=ei.dtype)])
    deg = np.zeros(7, np.float32)
    np.add.at(deg, dst, np.float32(1.0))
    dinv = np.where(deg > 0, deg ** np.float32(-0.5), np.float32(0.0)).astype(
        np.float32
    )
    w = (dinv[src] * dinv[dst]).astype(np.float32)
    A = np.zeros((7, 7), np.float32)
    np.add.at(A, (dst, src), w)
    return A


def _pack_weights(A, W1, W2, Wl1, Wl2, bl2):
    M1 = np.kron(A.T, np.asarray(W1)).astype(np.float32)  # [112, 56]
    M2 = np.kron(A.T, np.asarray(W2)).astype(np.float32)  # [56, 56]
    Wl1 = np.asarray(Wl1, np.float32)
    Wl2 = np.asarray(Wl2, np.float32)
    blob = np.zeros((128, WGT_COLS), np.float32)
    blob[0:112, 0:56] = M1
    blob[0:56, 64:120] = M2
    blob[64:120, 120:176] = M2
    blob[0:56, 176:200] = Wl1
    blob[56:112, 200:224] = Wl1
    blob[0:24, 240] = Wl2[:, 0]
    blob[24:48, 241] = Wl2[:, 0]
    blob[64:88, 242] = Wl2[:, 0]
    blob[88:112, 243] = Wl2[:, 0]
    # h3q row 48 is forced to 1.0 by the dve3 bias; bl2 rides in that row so
    # the L4 matmul adds it and the y pass is a pure copy.
    blob[48, 240:244] = np.float32(np.asarray(bl2).reshape(-1)[0])
    return blob.astype(np.float16)


def _pack_scalars(b1, b2, bl1):
    b1 = np.asarray(b1, np.float32)
    b2 = np.asarray(b2, np.float32)
    bl1 = np.asarray(bl1, np.float32)
    blob = np.zeros((128, SCL_COLS), np.float32)
    blob[0:56, 0] = np.tile(b1, 7)
    blob[64:120, 0] = np.tile(b1, 7)
    blob[0:56, 1] = np.tile(b2, 7)
    blob[56:112, 1] = np.tile(b2, 7)
    blob[0:24, 3] = bl1
    blob[24:48, 3] = bl1
    blob[64:88, 3] = bl1
    blob[88:112, 3] = bl1
    blob[48, 3] = np.float32(1.0)  # h3q ones-row (carries bl2 via Wl2q)
    return blob


def _split_multiwaits(nc):
    """Walrus accepts only one sync wait per lowered instruction; hoist all
    but the last wait of any multi-wait instruction onto single-wait NOPs
    placed immediately before it on the same engine."""
    for f in nc.m.functions:
        for bb in f.blocks:
            out = []
            changed = False
            for inst in bb.instructions:
                si = inst.sync_info
                if si is not None and si.on_wait and len(si.on_wait) > 1:
                    waits = list(si.on_wait)
                    for w in waits[:-1]:
                        nop = mybir.InstNoOp(
                            name=nc.get_next_instruction_name(),
                            engine=inst.engine,
                            sync_info=mybir.SyncInfo(on_wait=[w], on_update=[]),
                            text_hint="split_wait",
                            bass_nofuse=True,
                        )
                        out.append(nop)
                    inst.sync_info = mybir.SyncInfo(
                        on_wait=[waits[-1]], on_update=list(si.on_update or [])
                    )
                    changed = True
                out.append(inst)
            if changed:
                bb.instructions = out


def _build_nc():
    nc = bass.Bass("TRN2", target_bir_lowering=False, debug=False)
    xin = nc.dram_tensor("xin", [F_IN, BPC], F16, kind="ExternalInput").ap()
    wgt = nc.dram_tensor("wgt", [128, WGT_COLS], F16, kind="ExternalInput").ap()
    scl = nc.dram_tensor("scl", [128, SCL_COLS], F32, kind="ExternalInput").ap()
    yout = nc.dram_tensor("y", [100, (NQ // 4) * 512], F16,
                          kind="ExternalOutput").ap()

    relu = mybir.ActivationFunctionType.Relu
    add_op = mybir.AluOpType.add
    max_op = mybir.AluOpType.max

    with tile.TileContext(nc) as tc, ExitStack() as ctx:
        wpool = ctx.enter_context(tc.tile_pool(name="wpool", bufs=1))
        xpool = ctx.enter_context(tc.tile_pool(name="xpool", bufs=NTILE))
        ps_pool = ctx.enter_context(tc.tile_pool(name="ps", bufs=1, space="PSUM"))

        wb = wpool.tile([128, WGT_COLS], F16)
        nc.sync.dma_start(wb[:, :], wgt)
        sb = wpool.tile([128, SCL_COLS], F32)
        nc.sync.dma_start(sb[:, :], scl)

        w_m1 = wb[0:112, 0:64]
        w_2 = wb[0:128, 64:176]
        w_3 = wb[0:112, 176:240]
        w_4 = wb[0:128, 240:244]
        b1v = sb[0:128, 0]
        b2v = sb[0:112, 1]
        zf2 = sb[0:112, 2]
        b3v = sb[0:128, 3]
        zf3 = sb[0:128, 2]

        # engine warmups: pull ACT table load / engine spin-up into the DMA
        # window instead of blocking the first real pass
        wrm = wpool.tile([1, 2], F32)
        nc.scalar.activation(wrm[0:1, 0:1], wrm[0:1, 1:2], relu)
        nc.vector.tensor_copy(wrm[0:1, 0:1], wrm[0:1, 1:2])

        # persistent SBUF intermediates (written in disjoint slices)
        h1p = wpool.tile([128, NTILE * 512], F16)
        h2p = wpool.tile([112, NTILE * 512], F16)
        h3q = wpool.tile([128, NQ * 512], F16)
        ysb = wpool.tile([100, (NQ // 4) * 512], F16)

        # PSUM: 4 bank-pair tiles (single-buffered; the fused schedule keeps
        # ~a chunk of other-stage PE work between reuses) + 2 banks for y.
        p1t = ps_pool.tile([128, 1024], F32, name="p1", tag="p1")
        p2t = ps_pool.tile([112, 1024], F32, name="p2", tag="p2")
        p3t = ps_pool.tile([128, 1024], F32, name="p3", tag="p3")
        p4 = [ps_pool.tile([100, 512], F32, name=f"p4_{i}", tag=f"p4{i}")
              for i in range(2)]

        xt = {}
        NC_CHUNKS = NQ // CHUNK_Q  # chunk = 2 q's = 4 tiles = 4096 samples

        def l1gen(g):
            # tiles 2g, 2g+1 -> p1t halves
            for j, t in enumerate((2 * g, 2 * g + 1)):
                xt[t] = xpool.tile([F_IN, 1024], F16, name=f"xt{t}", tag="xt")
                nc.sync.dma_start(xt[t][:, :], xin[:, ds(t * 1024, 1024)])
                c = ds(j * 512, 512)
                nc.tensor.matmul(p1t[0:64, c], w_m1, xt[t][:, 0:512],
                                 start=True, stop=True, tile_position=(0, 0))
                nc.tensor.matmul(p1t[64:128, c], w_m1, xt[t][:, 512:1024],
                                 start=True, stop=True, tile_position=(0, 64))

        def act1(g):
            nc.scalar.activation(
                h1p[:, ds(2 * g * 512, 1024)], p1t[:, :], relu,
                bias=b1v[:, None],
            )
            xt.pop(2 * g, None)
            xt.pop(2 * g + 1, None)

        def l2gen(g):
            for j, t in enumerate((2 * g, 2 * g + 1)):
                nc.tensor.matmul(p2t[:, ds(j * 512, 512)], w_2,
                                 h1p[:, ds(t * 512, 512)],
                                 start=True, stop=True, tile_position=(0, 0))

        def dve2(g):
            nc.vector.tensor_scalar(
                h2p[:, ds(2 * g * 512, 1024)], p2t[:, :],
                b2v[:, None], zf2[:, None], add_op, max_op,
            )

        def l3gen(G):
            # q = 2G, 2G+1 -> p3t halves
            for j, q in enumerate((2 * G, 2 * G + 1)):
                c = ds(j * 512, 512)
                nc.tensor.matmul(p3t[0:64, c], w_3,
                                 h2p[:, ds(2 * q * 512, 512)],
                                 start=True, stop=True, tile_position=(0, 0))
                nc.tensor.matmul(p3t[64:128, c], w_3,
                                 h2p[:, ds((2 * q + 1) * 512, 512)],
                                 start=True, stop=True, tile_position=(0, 64))

        def act3(G):
            nc.scalar.activation(
                h3q[:, ds(2 * G * 512, 1024)], p3t[:, :], relu,
                bias=b3v[:, None],
            )

        def l4(q):
            # y of q at psum partitions 32*(q%4) + [0:4) of group q//4
            pos = 32 * (q % 4)
            nc.tensor.matmul(p4[(q // 4) % 2][pos:pos + 4, :], w_4,
                             h3q[:, ds(q * 512, 512)],
                             start=True, stop=True, tile_position=(0, pos))

        def dve_y(q):
            if q % 4 != 3:
                return
            G = q // 4
            nc.vector.tensor_copy(ysb[:, ds(G * 512, 512)], p4[G % 2][:, :])
            nc.gpsimd.dma_start(yout[:, ds(G * 512, 512)],
                                ysb[:, ds(G * 512, 512)])

        # Fused schedule, chunk C covers q-pair group C (2 q's / 4 tiles /
        # 4096 samples). Stages run chunk-lagged: L1(C), L2(C-1), L3(C-2),
        # L4(C-3). The emission order alternates stages so the PE always has
        # >= 2 matmuls in flight while a single-buffered psum drain runs.
        for C in range(NC_CHUNKS + 3):
            if C < NC_CHUNKS:
                l1gen(2 * C)
                act1(2 * C)
            if 1 <= C <= NC_CHUNKS:
                l2gen(2 * (C - 1))
                dve2(2 * (C - 1))
            if 2 <= C <= NC_CHUNKS + 1:
                l3gen(C - 2)
                act3(C - 2)
            if C < NC_CHUNKS:
                l1gen(2 * C + 1)
                act1(2 * C + 1)
            if 1 <= C <= NC_CHUNKS:
                l2gen(2 * (C - 1) + 1)
                dve2(2 * (C - 1) + 1)
            if 3 <= C <= NC_CHUNKS + 2:
                for q in (2 * (C - 3), 2 * (C - 3) + 1):
                    l4(q)
                    dve_y(q)

    _split_multiwaits(nc)
    return nc


_NC_CACHE = None


def _get_nc():
    global _NC_CACHE
    if _NC_CACHE is None:
        _NC_CACHE = _build_nc()
    return _NC_CACHE


def _pack_x(x1):
    """Per-core [BPC, 112] -> [112, BPC] fp16 with the tile permutation:
    sample 4q+m of L3-block Q lands in L1-tile t=2Q+(m//2) at column
    (m%2)*512 + (q - 512Q)."""
    x1 = np.asarray(x1, np.float32)
    out = []
    for c in range(N_CORES):
        xc = x1[c * BPC:(c + 1) * BPC]
        v = xc.reshape(NQ, 512, 2, 2, F_IN)
        xin = v.transpose(0, 2, 3, 1, 4).reshape(BPC, F_IN)
        out.append(np.ascontiguousarray(xin.T.astype(np.float16)))
    return out


def _make_in_maps(x1, edge_index, W1, b1, W2, b2, Wl1, bl1, Wl2, bl2):
    A = _norm_adj_np(edge_index)
    wgt = _pack_weights(A, W1, W2, Wl1, Wl2, bl2)
    scl = _pack_scalars(b1, b2, bl1)
    xs = _pack_x(x1)
    return [{"xin": xs[c], "wgt": wgt, "scl": scl} for c in range(N_CORES)]


def kernel(x1, edge_index, W1, b1, W2, b2, Wl1, bl1, Wl2, bl2, **_unused):
    in_maps = _make_in_maps(x1, edge_index, W1, b1, W2, b2, Wl1, bl1, Wl2, bl2)
    nc = _get_nc()
    res = run_bass_kernel_spmd(nc, in_maps, list(range(N_CORES)))
    return _gather_y(res.results)


def _gather_y(results):
    parts = []
    for c in range(N_CORES):
        yc = results[c]["y"].astype(np.float32)  # [100, (NQ//4)*512]
        out = np.empty(BPC, np.float32)
        for q in range(NQ):
            r = 32 * (q % 4)
            blk = yc[r:r + 4, (q // 4) * 512:(q // 4) * 512 + 512]
            out[q * 2048:(q + 1) * 2048] = blk.T.reshape(-1)
        parts.append(out)
    return np.concatenate(parts).reshape(B, 1).astype(np.float32)
